# revision 38
# baseline (speedup 1.0000x reference)
# DCN CrossLayer kernel for Trainium2 (8 NeuronCores, data-parallel over batch).
#
# Reference computation (per example row x of length D, L=3 layers):
#   cross = x
#   for i in range(L):
#       s_i   = <cross, W_i>                  (scalar per example)
#       cross = x * s_i + bias_i + cross
#
# Algebraic collapse: cross_i = a_i * x + B_i with per-example scalar a_i and
# batch-independent vector B_i = sum_{j<i} bias_j.  Then
#   s_i     = a_i * t_i + c_i,   t_i = <x, W_i>,  c_i = <B_i, W_i>
#   a_{i+1} = a_i * (1 + t_i) + c_i
#   out     = a_L * x + B_L
# so the device kernel only needs the three dot products t_i = <x, W_i>
# (one skinny matmul against W^T), a tiny per-row recurrence, and one
# per-row scale of x.  c_i and B_L are computed on the host.
#
# Schedule: the kernel is fabric-DMA bound (8.4 MiB in + 8.4 MiB out per
# core; the in and out streams share the SDMA fabric at ~420 GB/s combined,
# so ~40us of unavoidable DMA work + ~9us fixed NEFF startup + ~2.5us
# completion tail).  Compute is spread across engines so neither DMA stream
# ever waits on compute (each engine stays under ~27us):
#   - sync (SP HWDGE q1):   8 x 1 MiB fp32 in-DMAs, issued up front, whole
#                           shard SBUF-resident (no buffer-recycle stalls)
#   - PE:                   8 fp32 transposes per tile (transpose_mode is
#                           full rate for fp32 on trn2) + 8 accumulating
#                           fp16 dot-matmuls
#   - DVE:                  fused PSUM-fp32 -> SBUF-fp16 copy (the cast rides
#                           the copy), per-row recurrence
#   - ACT (scalar):         ys = x * a3 via activation(Copy, scale=a3) from
#                           the resident fp32 x (full output precision), plus
#                           8 x 1 MiB out-DMA triggers (ACT HWDGE q10) that
#                           interleave with the in stream at the SDMA rings
# Measured losses that resisted removal: ~7us NEFF preamble before the first
# DMA trigger, ~2.5us end-of-stream straggler trickle (per-SDMA-engine skew),
# ~2.4us final HBM write-receipt.  Variants that measured WORSE: host-permuted
# per-partition-contiguous layouts (HBM channel interleave loss), graded/
# smaller DMA chunks, gpsimd casts or recurrence (slow Q7 elementwise, extra
# cross-engine sync hops), split half-tile copies.
from contextlib import ExitStack, nullcontext

import numpy as np

import concourse.bacc as bacc
import concourse.bass as bass
import concourse.tile as tile
from concourse import mybir
from concourse.bass_utils import run_bass_kernel_spmd
from concourse.masks import make_identity

B, D, L = 16384, 1024, 3
N_CORES = 8
ROWS = B // N_CORES  # rows per core
P = 128
TILES = ROWS // P  # 16 row-tiles per core
KCH = D // P  # 8 d-chunks of 128

# tiles per DMA chunk; small edge chunks = early out-stream start + short
# final ring drain
IN_SCHED = [2, 2, 2, 2, 2, 2, 2, 2]
OUT_SCHED = [2, 2, 2, 2, 2, 2, 2, 2]
assert sum(IN_SCHED) == TILES and sum(OUT_SCHED) == TILES

F32 = mybir.dt.float32
F16 = mybir.dt.float16

# test.py can flip these before calling kernel() to get an NTFF profile.
TRACE = False
LAST_RESULT = None


def _build(has_bias: bool, c1: float, c2: float) -> bass.Bass:
    nc = bacc.Bacc("TRN2", target_bir_lowering=False)
    x = nc.dram_tensor("x", [ROWS, D], F32, kind="ExternalInput")
    wt = nc.dram_tensor("wt", [P, KCH, L], F16, kind="ExternalInput")
    if has_bias:
        b3 = nc.dram_tensor("b3", [1, D], F32, kind="ExternalInput")
    y = nc.dram_tensor("y", [ROWS, D], F32, kind="ExternalOutput")

    # row r = t*P + p  ->  [p][t][d]; the 4 KiB row interleave across
    # partitions spreads each DMA chunk over HBM channels (measurably faster
    # than a host-permuted per-partition-contiguous layout)
    xv = x.rearrange("(t p) d -> p t d", p=P)
    yv = y.rearrange("(t p) d -> p t d", p=P)

    with tile.TileContext(nc) as tc, ExitStack() as ctx:
        singles = ctx.enter_context(tc.tile_pool(name="singles", bufs=1))
        xpool = ctx.enter_context(tc.tile_pool(name="xpool", bufs=len(IN_SCHED)))
        xtpool = ctx.enter_context(tc.tile_pool(name="xtpool", bufs=4))
        opool = ctx.enter_context(tc.tile_pool(name="opool", bufs=4))
        small = ctx.enter_context(tc.tile_pool(name="small", bufs=8))
        psT = ctx.enter_context(tc.tile_pool(name="psT", bufs=3, space="PSUM"))
        psD = ctx.enter_context(tc.tile_pool(name="psD", bufs=2, space="PSUM"))

        # tiny constant DMA goes on the SWDGE ring so it cannot delay the
        # first big x in-DMA on the SP HWDGE ring
        wt_sb = singles.tile([P, KCH, L], F16)
        nc.gpsimd.dma_start(out=wt_sb, in_=wt[:])
        eye_sb = singles.tile([P, P], F32)
        make_identity(nc, eye_sb)
        # dummy activation: forces the 16KB ACT function-table DMA to load
        # NOW, before the big streams start — otherwise it lands mid-stream
        # on the same SDMA engine whose lag sets the kernel tail
        warm = singles.tile([P, 1], F32)
        nc.gpsimd.memset(warm, 0.0)
        nc.scalar.activation(warm, warm, mybir.ActivationFunctionType.Copy)

        if has_bias:
            b3_sb = singles.tile([P, D], F32)
            b3_bcast = bass.AP(
                tensor=b3.tensor, offset=b3.offset, ap=[[0, P], b3.ap[1]]
            )
            nc.gpsimd.dma_start(out=b3_sb, in_=b3_bcast)

        # all in-DMAs issued up front on the SP HWDGE ring; the whole fp32
        # shard stays resident so nothing downstream throttles the in stream
        tile_src = [None] * TILES  # tile idx -> (xs tile, slot within chunk)
        t_off = 0
        for n in IN_SCHED:
            xs = xpool.tile([P, n, D], F32, tag=f"xs{n}")
            nc.sync.dma_start(out=xs, in_=xv[:, t_off : t_off + n, :])
            for u in range(n):
                tile_src[t_off + u] = (xs, u)
            t_off += n

        t_off = 0
        for n in OUT_SCHED:
            ys = opool.tile([P, n, D], F32, tag=f"ys{n}")
            for u in range(n):
                xs, slot = tile_src[t_off + u]
                xrow = xs[:, slot, :]
                # transpose fp32 x tile: 8 x [128,128] blocks -> psum
                pxt = psT.tile([P, KCH, P], F32)
                for k in range(KCH):
                    nc.tensor.transpose(
                        pxt[:, k, :], xrow[:, k * P : (k + 1) * P], eye_sb
                    )
                # PSUM->SBUF copy doubles as the fp32->fp16 cast
                xt = xtpool.tile([P, KCH, P], F16)
                nc.vector.tensor_copy(xt, pxt)
                # t[row, l] = sum_d x[row, d] * W[l, d], accumulated over chunks
                pt = psD.tile([P, L], F32)
                for k in range(KCH):
                    nc.tensor.matmul(
                        pt,
                        xt[:, k, :],
                        wt_sb[:, k, :],
                        start=(k == 0),
                        stop=(k == KCH - 1),
                    )
                # a3 = ((1+t0)(1+t1)+c1)(1+t2)+c2
                ut = small.tile([P, L], F32, tag="ut")
                nc.vector.tensor_scalar_add(ut, pt, 1.0)
                a3 = small.tile([P, 1], F32, tag="a3")
                nc.vector.tensor_mul(a3, ut[:, 0:1], ut[:, 1:2])
                if c1 != 0.0:
                    nc.vector.tensor_scalar_add(a3, a3, c1)
                nc.vector.tensor_mul(a3, a3, ut[:, 2:3])
                if c2 != 0.0:
                    nc.vector.tensor_scalar_add(a3, a3, c2)
                # out = x * a3 (+ B_L) on the ACT engine
                nc.scalar.activation(
                    ys[:, u, :],
                    xrow,
                    mybir.ActivationFunctionType.Copy,
                    scale=a3,
                )
                if has_bias:
                    nc.vector.tensor_add(ys[:, u, :], ys[:, u, :], b3_sb)
            # out-DMA on the ACT HWDGE ring (q10), interleaves with in q1
            nc.scalar.dma_start(out=yv[:, t_off : t_off + n, :], in_=ys)
            t_off += n
    nc.finalize()
    return nc


def kernel(x, W, bias):
    global LAST_RESULT
    x2 = np.asarray(x, dtype=np.float32).reshape(B, D)
    W2 = np.asarray(W, dtype=np.float32).reshape(L, D)
    B2 = np.asarray(bias, dtype=np.float32).reshape(L, D)

    # host-side constants
    has_bias = bool(np.any(B2 != 0.0))
    c1 = float(B2[0] @ W2[1])
    c2 = float((B2[0] + B2[1]) @ W2[2])
    b3_host = np.ascontiguousarray(B2.sum(axis=0).reshape(1, D))
    # wt[p, k, l] = W[l, k*128 + p]
    wt_host = np.ascontiguousarray(
        W2.T.reshape(KCH, P, L).transpose(1, 0, 2).astype(np.float16)
    )

    nc = _build(has_bias, c1 if has_bias else 0.0, c2 if has_bias else 0.0)

    shards = np.split(np.ascontiguousarray(x2), N_CORES, axis=0)
    in_maps = []
    for c in range(N_CORES):
        m = {"x": shards[c], "wt": wt_host}
        if has_bias:
            m["b3"] = b3_host
        in_maps.append(m)

    kwargs = {}
    if TRACE:
        kwargs = dict(trace=True, trace_cores=[0])
    # the shared device occasionally throws a transient
    # NRT_EXEC_UNIT_UNRECOVERABLE; a plain retry recovers it
    last_err = None
    for _attempt in range(3):
        try:
            res = run_bass_kernel_spmd(
                nc, in_maps, core_ids=list(range(N_CORES)), **kwargs
            )
            break
        except Exception as e:  # noqa: BLE001
            last_err = e
    else:
        raise last_err
    LAST_RESULT = res
    out = np.concatenate([res.results[c]["y"] for c in range(N_CORES)], axis=0)
    return np.ascontiguousarray(out.reshape(B, D, 1))


# revision 41
# speedup vs baseline: 1.0088x; 1.0088x over previous
# DCN CrossLayer kernel for Trainium2 (8 NeuronCores, data-parallel over batch).
#
# Reference computation (per example row x of length D, L=3 layers):
#   cross = x
#   for i in range(L):
#       s_i   = <cross, W_i>                  (scalar per example)
#       cross = x * s_i + bias_i + cross
#
# Algebraic collapse: cross_i = a_i * x + B_i with per-example scalar a_i and
# batch-independent vector B_i = sum_{j<i} bias_j.  Then
#   s_i     = a_i * t_i + c_i,   t_i = <x, W_i>,  c_i = <B_i, W_i>
#   a_{i+1} = a_i * (1 + t_i) + c_i
#   out     = a_L * x + B_L
# so the device kernel only needs the three dot products t_i = <x, W_i>
# (one skinny matmul against W^T), a tiny per-row recurrence, and one
# per-row scale of x.  c_i and B_L are computed on the host.
#
# Schedule: the kernel is fabric-DMA bound (8.4 MiB in + 8.4 MiB out per
# core; the in and out streams share the SDMA fabric at ~420 GB/s combined,
# so ~40us of unavoidable DMA work + ~9us fixed NEFF startup + ~2.5us
# completion tail).  Compute is spread across engines so neither DMA stream
# ever waits on compute (each engine stays under ~27us):
#   - sync (SP HWDGE q1):   8 x 1 MiB fp32 in-DMAs, issued up front, whole
#                           shard SBUF-resident (no buffer-recycle stalls)
#   - PE:                   8 fp32 transposes per tile (transpose_mode is
#                           full rate for fp32 on trn2) + 8 accumulating
#                           fp16 dot-matmuls
#   - DVE:                  fused PSUM-fp32 -> SBUF-fp16 copy (the cast rides
#                           the copy), per-row recurrence
#   - ACT (scalar):         ys = x * a3 via activation(Copy, scale=a3) from
#                           the resident fp32 x (full output precision), plus
#                           8 x 1 MiB out-DMA triggers (ACT HWDGE q10) that
#                           interleave with the in stream at the SDMA rings
# Measured losses that resisted removal: ~7us NEFF preamble before the first
# DMA trigger, ~2.5us end-of-stream straggler trickle (per-SDMA-engine skew),
# ~2.4us final HBM write-receipt.  Variants that measured WORSE: host-permuted
# per-partition-contiguous layouts (HBM channel interleave loss), graded/
# smaller DMA chunks, gpsimd casts or recurrence (slow Q7 elementwise, extra
# cross-engine sync hops), split half-tile copies.
from contextlib import ExitStack, nullcontext

import numpy as np

import concourse.bacc as bacc
import concourse.bass as bass
import concourse.tile as tile
from concourse import mybir
from concourse.bass_utils import run_bass_kernel_spmd
from concourse.masks import make_identity

B, D, L = 16384, 1024, 3
N_CORES = 8
ROWS = B // N_CORES  # rows per core
P = 128
TILES = ROWS // P  # 16 row-tiles per core
KCH = D // P  # 8 d-chunks of 128

# tiles per DMA chunk; small edge chunks = early out-stream start + short
# final ring drain
IN_SCHED = [2, 2, 2, 2, 2, 2, 2, 2]
OUT_SCHED = [2, 2, 2, 2, 2, 2, 2, 2]
assert sum(IN_SCHED) == TILES and sum(OUT_SCHED) == TILES

F32 = mybir.dt.float32
F16 = mybir.dt.float16

# test.py can flip these before calling kernel() to get an NTFF profile.
TRACE = False
LAST_RESULT = None


def _build(has_bias: bool, c1: float, c2: float) -> bass.Bass:
    nc = bacc.Bacc("TRN2", target_bir_lowering=False)
    x = nc.dram_tensor("x", [ROWS, D], F32, kind="ExternalInput")
    wt = nc.dram_tensor("wt", [P, KCH, L], F16, kind="ExternalInput")
    if has_bias:
        b3 = nc.dram_tensor("b3", [1, D], F32, kind="ExternalInput")
    y = nc.dram_tensor("y", [ROWS, D], F32, kind="ExternalOutput")

    # row r = t*P + p  ->  [p][t][d]; the 4 KiB row interleave across
    # partitions spreads each DMA chunk over HBM channels (measurably faster
    # than a host-permuted per-partition-contiguous layout)
    xv = x.rearrange("(t p) d -> p t d", p=P)
    yv = y.rearrange("(t p) d -> p t d", p=P)

    with tile.TileContext(nc) as tc, ExitStack() as ctx:
        singles = ctx.enter_context(tc.tile_pool(name="singles", bufs=1))
        xpool = ctx.enter_context(tc.tile_pool(name="xpool", bufs=len(IN_SCHED)))
        xtpool = ctx.enter_context(tc.tile_pool(name="xtpool", bufs=4))
        opool = ctx.enter_context(tc.tile_pool(name="opool", bufs=4))
        small = ctx.enter_context(tc.tile_pool(name="small", bufs=8))
        psT = ctx.enter_context(tc.tile_pool(name="psT", bufs=3, space="PSUM"))
        psD = ctx.enter_context(tc.tile_pool(name="psD", bufs=2, space="PSUM"))

        # tiny constant DMA goes on the SWDGE ring so it cannot delay the
        # first big x in-DMA on the SP HWDGE ring
        wt_sb = singles.tile([P, KCH, L], F16)
        nc.gpsimd.dma_start(out=wt_sb, in_=wt[:])
        eye_sb = singles.tile([P, P], F32)
        make_identity(nc, eye_sb)

        if has_bias:
            b3_sb = singles.tile([P, D], F32)
            b3_bcast = bass.AP(
                tensor=b3.tensor, offset=b3.offset, ap=[[0, P], b3.ap[1]]
            )
            nc.gpsimd.dma_start(out=b3_sb, in_=b3_bcast)

        # all in-DMAs issued up front on the SP HWDGE ring; the whole fp32
        # shard stays resident so nothing downstream throttles the in stream
        tile_src = [None] * TILES  # tile idx -> (xs tile, slot within chunk)
        t_off = 0
        for n in IN_SCHED:
            xs = xpool.tile([P, n, D], F32, tag=f"xs{n}")
            nc.sync.dma_start(out=xs, in_=xv[:, t_off : t_off + n, :])
            for u in range(n):
                tile_src[t_off + u] = (xs, u)
            t_off += n

        t_off = 0
        for n in OUT_SCHED:
            ys = opool.tile([P, n, D], F32, tag=f"ys{n}")
            # both tiles of the chunk accumulate dots into one PSUM buffer so
            # the recurrence runs as single wide ops (fewer instructions =
            # less per-op overhead and a shorter program for the runtime's
            # mid-kernel instruction refills)
            pt = psD.tile([P, n, L], F32, tag=f"pt{n}")
            for u in range(n):
                xs, slot = tile_src[t_off + u]
                xrow = xs[:, slot, :]
                # transpose fp32 x tile: 8 x [128,128] blocks -> psum
                pxt = psT.tile([P, KCH, P], F32)
                for k in range(KCH):
                    nc.tensor.transpose(
                        pxt[:, k, :], xrow[:, k * P : (k + 1) * P], eye_sb
                    )
                # PSUM->SBUF copy doubles as the fp32->fp16 cast
                xt = xtpool.tile([P, KCH, P], F16)
                nc.vector.tensor_copy(xt, pxt)
                # t[row, u, l] = sum_d x[row, d] * W[l, d], over d-chunks
                for k in range(KCH):
                    nc.tensor.matmul(
                        pt[:, u, :],
                        xt[:, k, :],
                        wt_sb[:, k, :],
                        start=(k == 0),
                        stop=(k == KCH - 1),
                    )
            # a3 = ((1+t0)(1+t1)+c1)(1+t2)+c2 for all tiles of the chunk
            ut = small.tile([P, n, L], F32, tag=f"ut{n}")
            nc.vector.tensor_scalar_add(ut, pt, 1.0)
            a3 = small.tile([P, n], F32, tag=f"a3{n}")
            nc.vector.tensor_mul(a3, ut[:, :, 0], ut[:, :, 1])
            if c1 != 0.0:
                nc.vector.tensor_scalar_add(a3, a3, c1)
            nc.vector.tensor_mul(a3, a3, ut[:, :, 2])
            if c2 != 0.0:
                nc.vector.tensor_scalar_add(a3, a3, c2)
            for u in range(n):
                xs, slot = tile_src[t_off + u]
                # out = x * a3 (+ B_L) on the ACT engine
                nc.scalar.activation(
                    ys[:, u, :],
                    xs[:, slot, :],
                    mybir.ActivationFunctionType.Copy,
                    scale=a3[:, u : u + 1],
                )
                if has_bias:
                    nc.vector.tensor_add(ys[:, u, :], ys[:, u, :], b3_sb)
            # out-DMA on the ACT HWDGE ring (q10), interleaves with in q1
            nc.scalar.dma_start(out=yv[:, t_off : t_off + n, :], in_=ys)
            t_off += n
    nc.finalize()
    return nc


def kernel(x, W, bias):
    global LAST_RESULT
    x2 = np.asarray(x, dtype=np.float32).reshape(B, D)
    W2 = np.asarray(W, dtype=np.float32).reshape(L, D)
    B2 = np.asarray(bias, dtype=np.float32).reshape(L, D)

    # host-side constants
    has_bias = bool(np.any(B2 != 0.0))
    c1 = float(B2[0] @ W2[1])
    c2 = float((B2[0] + B2[1]) @ W2[2])
    b3_host = np.ascontiguousarray(B2.sum(axis=0).reshape(1, D))
    # wt[p, k, l] = W[l, k*128 + p]
    wt_host = np.ascontiguousarray(
        W2.T.reshape(KCH, P, L).transpose(1, 0, 2).astype(np.float16)
    )

    nc = _build(has_bias, c1 if has_bias else 0.0, c2 if has_bias else 0.0)

    shards = np.split(np.ascontiguousarray(x2), N_CORES, axis=0)
    in_maps = []
    for c in range(N_CORES):
        m = {"x": shards[c], "wt": wt_host}
        if has_bias:
            m["b3"] = b3_host
        in_maps.append(m)

    kwargs = {}
    if TRACE:
        kwargs = dict(trace=True, trace_cores=[0])
    # the shared device occasionally throws a transient
    # NRT_EXEC_UNIT_UNRECOVERABLE; a plain retry recovers it
    last_err = None
    for _attempt in range(3):
        try:
            res = run_bass_kernel_spmd(
                nc, in_maps, core_ids=list(range(N_CORES)), **kwargs
            )
            break
        except Exception as e:  # noqa: BLE001
            last_err = e
    else:
        raise last_err
    LAST_RESULT = res
    out = np.concatenate([res.results[c]["y"] for c in range(N_CORES)], axis=0)
    return np.ascontiguousarray(out.reshape(B, D, 1))


# revision 42
# speedup vs baseline: 1.0574x; 1.0482x over previous
# DCN CrossLayer kernel for Trainium2 (8 NeuronCores, data-parallel over batch).
#
# Reference computation (per example row x of length D, L=3 layers):
#   cross = x
#   for i in range(L):
#       s_i   = <cross, W_i>                  (scalar per example)
#       cross = x * s_i + bias_i + cross
#
# Algebraic collapse: cross_i = a_i * x + B_i with per-example scalar a_i and
# batch-independent vector B_i = sum_{j<i} bias_j.  Then
#   s_i     = a_i * t_i + c_i,   t_i = <x, W_i>,  c_i = <B_i, W_i>
#   a_{i+1} = a_i * (1 + t_i) + c_i
#   out     = a_L * x + B_L
# so the device kernel only needs the three dot products t_i = <x, W_i>
# (one skinny matmul against W^T), a tiny per-row recurrence, and one
# per-row scale of x.  c_i and B_L are computed on the host.
#
# Schedule: the kernel is fabric-DMA bound (8.4 MiB in + 8.4 MiB out per
# core; the in and out streams share the SDMA fabric at ~420 GB/s combined,
# so ~40us of unavoidable DMA work + ~9us fixed NEFF startup + ~2.5us
# completion tail).  Compute is spread across engines so neither DMA stream
# ever waits on compute (each engine stays under ~27us):
#   - sync (SP HWDGE q1):   8 x 1 MiB fp32 in-DMAs, issued up front, whole
#                           shard SBUF-resident (no buffer-recycle stalls)
#   - PE:                   8 fp32 transposes per tile (transpose_mode is
#                           full rate for fp32 on trn2) + 8 accumulating
#                           fp16 dot-matmuls
#   - DVE:                  fused PSUM-fp32 -> SBUF-fp16 copy (the cast rides
#                           the copy), per-row recurrence
#   - ACT (scalar):         ys = x * a3 via activation(Copy, scale=a3) from
#                           the resident fp32 x (full output precision), plus
#                           8 x 1 MiB out-DMA triggers (ACT HWDGE q10) that
#                           interleave with the in stream at the SDMA rings
# Measured losses that resisted removal: ~7us NEFF preamble before the first
# DMA trigger, ~2.5us end-of-stream straggler trickle (per-SDMA-engine skew),
# ~2.4us final HBM write-receipt.  Variants that measured WORSE: host-permuted
# per-partition-contiguous layouts (HBM channel interleave loss), graded/
# smaller DMA chunks, gpsimd casts or recurrence (slow Q7 elementwise, extra
# cross-engine sync hops), split half-tile copies.
from contextlib import ExitStack, nullcontext

import numpy as np

import concourse.bacc as bacc
import concourse.bass as bass
import concourse.tile as tile
from concourse import mybir
from concourse.bass_utils import run_bass_kernel_spmd
from concourse.masks import make_identity

B, D, L = 16384, 1024, 3
N_CORES = 8
ROWS = B // N_CORES  # rows per core
P = 128
TILES = ROWS // P  # 16 row-tiles per core
KCH = D // P  # 8 d-chunks of 128

# tiles per DMA chunk; small edge chunks = early out-stream start + short
# final ring drain
IN_SCHED = [2, 2, 2, 2, 2, 2, 2, 2]
OUT_SCHED = [2, 2, 2, 2, 2, 2, 2, 2]
assert sum(IN_SCHED) == TILES and sum(OUT_SCHED) == TILES

F32 = mybir.dt.float32
F16 = mybir.dt.float16

# test.py can flip these before calling kernel() to get an NTFF profile.
TRACE = False
LAST_RESULT = None


def _build(has_bias: bool, c1: float, c2: float) -> bass.Bass:
    nc = bacc.Bacc("TRN2", target_bir_lowering=False)
    x = nc.dram_tensor("x", [ROWS, D], F32, kind="ExternalInput")
    wt = nc.dram_tensor("wt", [P, KCH, L], F16, kind="ExternalInput")
    if has_bias:
        b3 = nc.dram_tensor("b3", [1, D], F32, kind="ExternalInput")
    y = nc.dram_tensor("y", [ROWS, D], F32, kind="ExternalOutput")

    # row r = t*P + p  ->  [p][t][d]; the 4 KiB row interleave across
    # partitions spreads each DMA chunk over HBM channels (measurably faster
    # than a host-permuted per-partition-contiguous layout)
    xv = x.rearrange("(t p) d -> p t d", p=P)
    yv = y.rearrange("(t p) d -> p t d", p=P)

    with tile.TileContext(nc) as tc, ExitStack() as ctx:
        singles = ctx.enter_context(tc.tile_pool(name="singles", bufs=1))
        xpool = ctx.enter_context(tc.tile_pool(name="xpool", bufs=len(IN_SCHED)))
        xtpool = ctx.enter_context(tc.tile_pool(name="xtpool", bufs=4))
        opool = ctx.enter_context(tc.tile_pool(name="opool", bufs=4))
        small = ctx.enter_context(tc.tile_pool(name="small", bufs=8))
        psT = ctx.enter_context(tc.tile_pool(name="psT", bufs=3, space="PSUM"))
        psD = ctx.enter_context(tc.tile_pool(name="psD", bufs=2, space="PSUM"))

        # tiny constant DMA goes on the SWDGE ring so it cannot delay the
        # first big x in-DMA on the SP HWDGE ring
        wt_sb = singles.tile([P, KCH, L], F16)
        nc.gpsimd.dma_start(out=wt_sb, in_=wt[:])
        eye_sb = singles.tile([P, P], F32)
        make_identity(nc, eye_sb)

        if has_bias:
            b3_sb = singles.tile([P, D], F32)
            b3_bcast = bass.AP(
                tensor=b3.tensor, offset=b3.offset, ap=[[0, P], b3.ap[1]]
            )
            nc.gpsimd.dma_start(out=b3_sb, in_=b3_bcast)

        # all in-DMAs issued up front on the SP HWDGE ring; the whole fp32
        # shard stays resident so nothing downstream throttles the in stream
        tile_src = [None] * TILES  # tile idx -> (xs tile, slot within chunk)
        t_off = 0
        for n in IN_SCHED:
            xs = xpool.tile([P, n, D], F32, tag=f"xs{n}")
            nc.sync.dma_start(out=xs, in_=xv[:, t_off : t_off + n, :])
            for u in range(n):
                tile_src[t_off + u] = (xs, u)
            t_off += n

        t_off = 0
        for n in OUT_SCHED:
            ys = opool.tile([P, n, D], F32, tag=f"ys{n}")
            for u in range(n):
                xs, slot = tile_src[t_off + u]
                xrow = xs[:, slot, :]
                # transpose fp32 x tile: 8 x [128,128] blocks -> psum
                pxt = psT.tile([P, KCH, P], F32)
                for k in range(KCH):
                    nc.tensor.transpose(
                        pxt[:, k, :], xrow[:, k * P : (k + 1) * P], eye_sb
                    )
                # PSUM->SBUF copy doubles as the fp32->fp16 cast
                xt = xtpool.tile([P, KCH, P], F16)
                nc.vector.tensor_copy(xt, pxt)
                # t[row, l] = sum_d x[row, d] * W[l, d], accumulated over chunks
                pt = psD.tile([P, L], F32)
                for k in range(KCH):
                    nc.tensor.matmul(
                        pt,
                        xt[:, k, :],
                        wt_sb[:, k, :],
                        start=(k == 0),
                        stop=(k == KCH - 1),
                    )
                # a3 = ((1+t0)(1+t1)+c1)(1+t2)+c2
                ut = small.tile([P, L], F32, tag="ut")
                nc.vector.tensor_scalar_add(ut, pt, 1.0)
                a3 = small.tile([P, 1], F32, tag="a3")
                nc.vector.tensor_mul(a3, ut[:, 0:1], ut[:, 1:2])
                if c1 != 0.0:
                    nc.vector.tensor_scalar_add(a3, a3, c1)
                nc.vector.tensor_mul(a3, a3, ut[:, 2:3])
                if c2 != 0.0:
                    nc.vector.tensor_scalar_add(a3, a3, c2)
                # out = x * a3 (+ B_L) on the ACT engine
                nc.scalar.activation(
                    ys[:, u, :],
                    xrow,
                    mybir.ActivationFunctionType.Copy,
                    scale=a3,
                )
                if has_bias:
                    nc.vector.tensor_add(ys[:, u, :], ys[:, u, :], b3_sb)
            # out-DMA on the ACT HWDGE ring (q10), interleaves with in q1
            nc.scalar.dma_start(out=yv[:, t_off : t_off + n, :], in_=ys)
            t_off += n
    nc.finalize()
    return nc


def kernel(x, W, bias):
    global LAST_RESULT
    x2 = np.asarray(x, dtype=np.float32).reshape(B, D)
    W2 = np.asarray(W, dtype=np.float32).reshape(L, D)
    B2 = np.asarray(bias, dtype=np.float32).reshape(L, D)

    # host-side constants
    has_bias = bool(np.any(B2 != 0.0))
    c1 = float(B2[0] @ W2[1])
    c2 = float((B2[0] + B2[1]) @ W2[2])
    b3_host = np.ascontiguousarray(B2.sum(axis=0).reshape(1, D))
    # wt[p, k, l] = W[l, k*128 + p]
    wt_host = np.ascontiguousarray(
        W2.T.reshape(KCH, P, L).transpose(1, 0, 2).astype(np.float16)
    )

    nc = _build(has_bias, c1 if has_bias else 0.0, c2 if has_bias else 0.0)

    shards = np.split(np.ascontiguousarray(x2), N_CORES, axis=0)
    in_maps = []
    for c in range(N_CORES):
        m = {"x": shards[c], "wt": wt_host}
        if has_bias:
            m["b3"] = b3_host
        in_maps.append(m)

    kwargs = {}
    if TRACE:
        kwargs = dict(trace=True, trace_cores=[0])
    # the shared device occasionally throws a transient
    # NRT_EXEC_UNIT_UNRECOVERABLE; a plain retry recovers it
    last_err = None
    for _attempt in range(3):
        try:
            res = run_bass_kernel_spmd(
                nc, in_maps, core_ids=list(range(N_CORES)), **kwargs
            )
            break
        except Exception as e:  # noqa: BLE001
            last_err = e
    else:
        raise last_err
    LAST_RESULT = res
    out = np.concatenate([res.results[c]["y"] for c in range(N_CORES)], axis=0)
    return np.ascontiguousarray(out.reshape(B, D, 1))
